# revision 9
# baseline (speedup 1.0000x reference)
"""Trainium2 Bass kernel: sampled logistic-regression forward.

reference math (per data row i, sample s):
    mean_i = X[i] . w_mu
    var_i  = sum_d X[i,d]^2 * exp(w_log_var[d])
    out[i,s] = sigmoid( sqrt(var_i) * z[s] + mean_i )

Full shapes: X [500000, 64], w_mu [64], w_log_var [64], z [128]
Output: [500000, 128] fp32.

Sharding: data-parallel over 8 NeuronCores, 62500 rows each.

Per-core pipeline (all fp32):
  - X streamed in chunks of 32 tiles x [125 rows, 64].
  - ACT: X2 = Square(X)            (sigmoid_and_others table set)
  - GPSIMD: A = X * w_mu_rep       (broadcasted weights, host-tiled)
  - DVE: V = X2 * exp(lv)_rep  (in-place), reduce(A) -> mean, reduce(V) -> var
  - DVE: Newton-Raphson rsqrt (bit-trick seed, 2 iters) -> y; std = var*y
  - mean/std written interleaved into a [125, 2*64] per-block stats tile
  - PE: transpose stats block -> [128, 125]; K=128 matmuls vs a constant
    block-diagonal Z2BIG [128, 64*128] give affine = mean + std*z in PSUM
  - ACT: Sigmoid [125, 512] PSUM->SBUF (batches of 4 tiles)
  - DMA out per 64-tile block.
"""

import os
from contextlib import ExitStack

import numpy as np

import concourse.bacc as bacc
import concourse.bass as bass
import concourse.tile as tile
from concourse import mybir
from concourse.bass_utils import run_bass_kernel_spmd

N_CORES = 8
D = 64
NS = 128
P = 125          # rows per tile (partition dim)
CH_T = 32        # tiles per chunk (DMA/compute granularity)
BLK_T = 64       # tiles per block (stats transpose granularity), = 2 chunks
SIG_T = 4        # tiles per sigmoid ACT op (4*128 = 512 = one PSUM bank)

RSQRT_MAGIC = 0x5F3759DF
F32 = mybir.dt.float32
U32 = mybir.dt.uint32


def build_program(rows: int, nrep: int = 1):
    """Build the single-core Bass/Tile program for `rows` rows (SPMD across cores).

    nrep > 1 repeats the whole streaming body (for timing: per-iteration
    device time = (t(nrep=k) - t(nrep=1)) / (k - 1)).
    """
    assert rows % P == 0
    ntiles = rows // P
    assert ntiles % SIG_T == 0

    nc = bacc.Bacc(
        "TRN2",
        target_bir_lowering=False,
        debug=False,
        num_devices=N_CORES,
    )

    x = nc.dram_tensor("x", [rows, D], F32, kind="ExternalInput")
    wmu_rep = nc.dram_tensor("wmu_rep", [P, CH_T * D], F32, kind="ExternalInput")
    elv_rep = nc.dram_tensor("elv_rep", [P, CH_T * D], F32, kind="ExternalInput")
    z2big = nc.dram_tensor("z2big", [2 * BLK_T, BLK_T * NS], F32, kind="ExternalInput")
    ident = nc.dram_tensor("ident", [P, P], F32, kind="ExternalInput")
    out = nc.dram_tensor("out", [rows, NS], F32, kind="ExternalOutput")

    xr = x.rearrange("(t p) d -> p t d", p=P)        # [125, ntiles, 64]
    outr = out.rearrange("(t p) s -> p t s", p=P)    # [125, ntiles, 128]

    nblocks = (ntiles + BLK_T - 1) // BLK_T

    with tile.TileContext(nc) as tc, ExitStack() as ctx:
        singles = ctx.enter_context(tc.tile_pool(name="singles", bufs=1))
        xin = ctx.enter_context(tc.tile_pool(name="xin", bufs=3))
        sqp = ctx.enter_context(tc.tile_pool(name="sqp", bufs=2))
        amp = ctx.enter_context(tc.tile_pool(name="amp", bufs=2))
        statp = ctx.enter_context(tc.tile_pool(name="statp", bufs=2))
        smalls = ctx.enter_context(tc.tile_pool(name="smalls", bufs=4))
        s2p = ctx.enter_context(tc.tile_pool(name="s2p", bufs=2))
        outp = ctx.enter_context(tc.tile_pool(name="outp", bufs=2))
        pst_pool = ctx.enter_context(tc.tile_pool(name="pst", bufs=2, space="PSUM"))
        paff_pool = ctx.enter_context(tc.tile_pool(name="paff", bufs=4, space="PSUM"))

        # one-time loads. Broadcast constants are "landed" on their consumer
        # engine via a copy so downstream tensor_tensor ops carry at most one
        # cross-engine sync wait (the walrus TT struct has a single wait slot).
        wmu_stage = singles.tile([P, CH_T, D], F32)
        nc.sync.dma_start(out=wmu_stage, in_=wmu_rep.rearrange("p (t d) -> p t d", d=D))
        wmu_sb = singles.tile([P, CH_T, D], F32)
        nc.vector.tensor_copy(wmu_sb, wmu_stage)
        elv_stage = singles.tile([P, CH_T, D], F32)
        nc.sync.dma_start(out=elv_stage, in_=elv_rep.rearrange("p (t d) -> p t d", d=D))
        elv_sb = singles.tile([P, CH_T, D], F32)
        nc.gpsimd.tensor_copy(elv_sb, elv_stage)
        z2_sb = singles.tile([2 * BLK_T, BLK_T * NS], F32)
        nc.sync.dma_start(out=z2_sb, in_=z2big[:, :])
        id_stage = singles.tile([P, P], F32)
        nc.sync.dma_start(out=id_stage, in_=ident[:, :])
        id_sb = singles.tile([P, P], F32)
        nc.vector.tensor_copy(id_sb, id_stage)
        magic_sb = singles.tile([P, CH_T], U32)
        nc.vector.memset(magic_sb, RSQRT_MAGIC)
        one_sb = singles.tile([P, 1], U32)
        nc.vector.memset(one_sb, 1)

        for _rep in range(nrep):
          for b in range(nblocks):
            nb = min(BLK_T, ntiles - b * BLK_T)   # tiles in this block
            tb = 2 * nb
            statblk = statp.tile([P, BLK_T, 2], F32)  # interleaved (mean, std)

            for cc in range(2):
                t0 = b * BLK_T + cc * CH_T        # global tile index
                if t0 >= ntiles:
                    break
                T = min(CH_T, ntiles - t0)
                lo = cc * CH_T                     # block-local tile offset

                xt = xin.tile([P, CH_T, D], F32)
                nc.sync.dma_start(out=xt[:, :T, :], in_=xr[:, t0 : t0 + T, :])

                # X^2 on ACT (Square lives in the sigmoid table set)
                x2 = sqp.tile([P, CH_T, D], F32)
                nc.scalar.activation(
                    out=x2[:, :T, :], in_=xt[:, :T, :],
                    func=mybir.ActivationFunctionType.Square,
                )
                # A = X * w_mu (broadcasted) on DVE (tolerates multi-queue DMA waits)
                at = amp.tile([P, CH_T, D], F32)
                nc.vector.tensor_mul(at[:, :T, :], xt[:, :T, :], wmu_sb[:, :T, :])
                # V = X^2 * exp(lv) in place on GPSIMD (single-producer input)
                nc.gpsimd.tensor_mul(x2[:, :T, :], x2[:, :T, :], elv_sb[:, :T, :])

                # mean -> statblk[:, lo:lo+T, 0]
                nc.vector.tensor_reduce(
                    out=statblk[:, lo : lo + T, 0],
                    in_=at[:, :T, :],
                    axis=mybir.AxisListType.X,
                    op=mybir.AluOpType.add,
                )
                # var (dense scratch)
                var = smalls.tile([P, CH_T], F32)
                nc.vector.tensor_reduce(
                    out=var[:, :T],
                    in_=x2[:, :T, :],
                    axis=mybir.AxisListType.X,
                    op=mybir.AluOpType.add,
                )

                # y = rsqrt(var): seed via 0x5f3759df - (bits >> 1), 2 NR iters
                vb = var[:, :T].bitcast(U32)
                yb = smalls.tile([P, CH_T], U32)
                nc.vector.tensor_scalar(
                    yb[:, :T], vb, one_sb[:, 0:1], None,
                    op0=mybir.AluOpType.logical_shift_right,
                )
                nc.vector.scalar_tensor_tensor(
                    out=yb[:, :T],
                    in0=magic_sb[:, :T],
                    scalar=0,
                    in1=yb[:, :T],
                    op0=mybir.AluOpType.bypass,
                    op1=mybir.AluOpType.subtract,
                )
                y = yb.bitcast(F32)
                t2 = smalls.tile([P, CH_T], F32)
                for _ in range(2):
                    nc.vector.tensor_mul(t2[:, :T], y[:, :T], y[:, :T])
                    nc.vector.tensor_mul(t2[:, :T], t2[:, :T], var[:, :T])
                    nc.vector.tensor_scalar(
                        t2[:, :T], t2[:, :T], -0.5, 1.5,
                        op0=mybir.AluOpType.mult,
                        op1=mybir.AluOpType.add,
                    )
                    nc.vector.tensor_mul(y[:, :T], y[:, :T], t2[:, :T])
                # std = var * y -> statblk[:, lo:lo+T, 1]
                nc.vector.tensor_mul(
                    statblk[:, lo : lo + T, 1], var[:, :T], y[:, :T]
                )

            # transpose stats block: [125, tb] -> [tb, 125] (PSUM), copy to SBUF
            pst = pst_pool.tile([2 * BLK_T, P], F32)
            nc.tensor.transpose(
                out=pst[:tb, :],
                in_=statblk.rearrange("p t two -> p (t two)")[:, :tb],
                identity=id_sb,
            )
            s2 = s2p.tile([2 * BLK_T, P], F32)
            nc.scalar.copy(out=s2[:tb, :], in_=pst[:tb, :])

            # affine (mean + std*z) via PE, sigmoid via ACT, batched out DMA
            outb = outp.tile([P, BLK_T, NS], F32)
            for g in range(nb // SIG_T):
                pa = paff_pool.tile([P, SIG_T * NS], F32)
                nc.tensor.matmul(
                    pa,
                    lhsT=s2[:tb, :],
                    rhs=z2_sb[:tb, g * SIG_T * NS : (g + 1) * SIG_T * NS],
                    start=True,
                    stop=True,
                )
                nc.scalar.activation(
                    out=outb[:, g * SIG_T : (g + 1) * SIG_T, :].rearrange(
                        "p t s -> p (t s)"
                    ),
                    in_=pa,
                    func=mybir.ActivationFunctionType.Sigmoid,
                )
            nc.sync.dma_start(
                out=outr[:, b * BLK_T : b * BLK_T + nb, :], in_=outb[:, :nb, :]
            )

    nc.finalize()
    return nc


def _host_consts(w_mu: np.ndarray, w_log_var: np.ndarray, z: np.ndarray):
    elv = np.exp(w_log_var.astype(np.float32))
    wmu_rep = np.tile(w_mu.astype(np.float32)[None, :], (P, CH_T)).reshape(P, CH_T * D)
    elv_rep = np.tile(elv[None, :], (P, CH_T)).reshape(P, CH_T * D)
    z2big = np.zeros((2 * BLK_T, BLK_T * NS), dtype=np.float32)
    for j in range(BLK_T):
        z2big[2 * j, j * NS : (j + 1) * NS] = 1.0
        z2big[2 * j + 1, j * NS : (j + 1) * NS] = z.astype(np.float32)
    ident = np.eye(P, dtype=np.float32)
    return wmu_rep, elv_rep, z2big, ident


_PROGRAM_CACHE: dict[int, "bass.Bass"] = {}


def run(X, w_mu, w_log_var, z, trace=False):
    X = np.ascontiguousarray(X, dtype=np.float32)
    n = X.shape[0]
    assert n % N_CORES == 0
    rows = n // N_CORES
    if rows not in _PROGRAM_CACHE:
        _PROGRAM_CACHE[rows] = build_program(rows)
    nc = _PROGRAM_CACHE[rows]

    wmu_rep, elv_rep, z2big, ident = _host_consts(
        np.asarray(w_mu), np.asarray(w_log_var), np.asarray(z)
    )
    in_maps = [
        {
            "x": X[i * rows : (i + 1) * rows],
            "wmu_rep": wmu_rep,
            "elv_rep": elv_rep,
            "z2big": z2big,
            "ident": ident,
        }
        for i in range(N_CORES)
    ]
    res = run_bass_kernel_spmd(nc, in_maps, list(range(N_CORES)), trace=trace)
    outs = [res.results[i]["out"] for i in range(N_CORES)]
    full = np.concatenate(outs, axis=0)
    return full, res


def kernel(X, w_mu, w_log_var, z):
    full, _ = run(X, w_mu, w_log_var, z, trace=False)
    return full
